# revision 3
# baseline (speedup 1.0000x reference)
"""Trainium2 Bass kernel for nn_AttentionEncoderLayer (B=4, C=8, F=2048, H=512, NH=8).

The reference's reshapes are pure buffer reinterpretations, so the whole layer is
independent per flat row n = f*B + b (8192 rows). Sharding: 1024 rows per core
across 8 cores, weights replicated, no collectives.

Device layout strategy:
  - Activations are ROW-major ([row-partition, feature-free]) everywhere the
    per-row math happens (softmax, LN, residuals).
  - Matmuls use the activation-transposed block as the stationary lhsT and the
    (pre-transposed) weight as the moving rhs: out = lhsT.T @ rhs lands
    ROW-major in PSUM directly.
  - q/k/v are pre-transposed (and bf16-cast) on the host into per-tile
    feature-major blocks, so the device never transposes the bulk inputs.
  - The three small on-device transposes (x -> O-proj, h1 -> FFN1, relu -> FFN2)
    go through the DMA xbar transpose engine (bf16), costing no compute-engine
    time.
"""

import numpy as np

# Problem constants (hardcoded per harness contract)
B, F, H = 4, 2048, 512
C = 8
NH = 8
DH = H // NH          # 64
HF = 2 * H            # 1024
NCORES = 8
NROWS = B * F         # 8192
RPC = NROWS // NCORES  # 1024 rows per core
P = 128
NT = RPC // P         # 8 row-tiles per core
IB = H // P           # 4 i-blocks of 128
JB = HF // P          # 8 j-blocks of 128
EPS = 1e-5

_CACHE = {}


def _build(nc_flags):
    """Build the Bass graph for one core's 1024 rows. nc_flags is a tuple of
    booleans: (bias_q, bias_k, bias_v, bias_o, bias_1, bias_2, ln1_aff, ln2_aff)."""
    import concourse.bass as bass
    import concourse.bacc as bacc
    import concourse.tile as tile
    from concourse import mybir

    (use_bq, use_bk, use_bv, use_bo, use_b1, use_b2, ln1_aff, ln2_aff) = nc_flags
    f32 = mybir.dt.float32
    bf16 = mybir.dt.bfloat16
    AF = mybir.ActivationFunctionType
    OP = mybir.AluOpType

    nc = bacc.Bacc("TRN2", target_bir_lowering=False)

    # ---- DRAM tensors (per-core shards; host-prepared layouts) ----
    qT_d = nc.dram_tensor("qT", [NT, P, IB, P], bf16, kind="ExternalInput")
    q32_d = nc.dram_tensor("q32", [NT, P, H], f32, kind="ExternalInput")
    kT_d = nc.dram_tensor("kT", [NT, P, C, IB, P], bf16, kind="ExternalInput")
    vT_d = nc.dram_tensor("vT", [NT, P, C, IB, P], bf16, kind="ExternalInput")
    wq_d = nc.dram_tensor("wq", [P, IB, H], bf16, kind="ExternalInput")
    wk_d = nc.dram_tensor("wk", [P, IB, H], bf16, kind="ExternalInput")
    wv_d = nc.dram_tensor("wv", [P, IB, H], bf16, kind="ExternalInput")
    wo_d = nc.dram_tensor("wo", [P, IB, H], bf16, kind="ExternalInput")
    w1_d = nc.dram_tensor("w1", [P, IB, HF], bf16, kind="ExternalInput")
    w2_d = nc.dram_tensor("w2", [P, JB, H], bf16, kind="ExternalInput")
    # biases / ln params shipped only when non-trivial
    bias_d = {}
    for name, used, width in (
        ("bq", use_bq, H), ("bk", use_bk, H), ("bv", use_bv, H),
        ("bo", use_bo, H), ("b1", use_b1, HF), ("b2", use_b2, H),
    ):
        if used:
            bias_d[name] = nc.dram_tensor(name, [1, width], bf16, kind="ExternalInput")
    if ln1_aff:
        ln1g_d = nc.dram_tensor("ln1g", [1, H], f32, kind="ExternalInput")
        ln1b_d = nc.dram_tensor("ln1b", [1, H], f32, kind="ExternalInput")
    if ln2_aff:
        ln2g_d = nc.dram_tensor("ln2g", [1, H], f32, kind="ExternalInput")
        ln2b_d = nc.dram_tensor("ln2b", [1, H], f32, kind="ExternalInput")
    out_d = nc.dram_tensor("out", [NT, P, H], f32, kind="ExternalOutput")

    any_bias = any([use_bq, use_bk, use_bv, use_bo, use_b1, use_b2])

    with tile.TileContext(nc) as tc:
        with (
            tc.tile_pool(name="wpool", bufs=1) as wpool,
            tc.tile_pool(name="inpool", bufs=2) as inpool,
            tc.tile_pool(name="proj", bufs=2) as proj,
            tc.tile_pool(name="small", bufs=2) as small,
            tc.tile_pool(name="psum", bufs=6, space="PSUM") as pspool,
        ):
            # ---- one-time weight loads ----
            wq_s = wpool.tile([P, IB, H], bf16, tag="wq")
            wk_s = wpool.tile([P, IB, H], bf16, tag="wk")
            wv_s = wpool.tile([P, IB, H], bf16, tag="wv")
            wo_s = wpool.tile([P, IB, H], bf16, tag="wo")
            w1_s = wpool.tile([P, IB, HF], bf16, tag="w1")
            w2_s = wpool.tile([P, JB, H], bf16, tag="w2")
            nc.sync.dma_start(wq_s[:], wq_d[:])
            nc.sync.dma_start(wk_s[:], wk_d[:])
            nc.sync.dma_start(wv_s[:], wv_d[:])
            nc.sync.dma_start(wo_s[:], wo_d[:])
            nc.sync.dma_start(w1_s[:], w1_d[:])
            nc.sync.dma_start(w2_s[:], w2_d[:])
            zero_s = wpool.tile([P, 1], f32, tag="zero")
            nc.vector.memset(zero_s[:], 0.0)
            eps_s = wpool.tile([P, 1], f32, tag="eps")
            nc.vector.memset(eps_s[:], EPS)
            bias_s = {}
            if any_bias:
                ones_s = wpool.tile([1, P], bf16, tag="ones")
                nc.vector.memset(ones_s[:], 1.0)
                for name, t in bias_d.items():
                    bs = wpool.tile([1, t.shape[1]], bf16, tag=name)
                    nc.sync.dma_start(bs[:], t[:])
                    bias_s[name] = bs
            if ln1_aff:
                ln1g_s = wpool.tile([P, H], f32, tag="ln1g")
                ln1b_s = wpool.tile([P, H], f32, tag="ln1b")
                nc.sync.dma_start(ln1g_s[:], ln1g_d[:].to_broadcast((P, H)))
                nc.sync.dma_start(ln1b_s[:], ln1b_d[:].to_broadcast((P, H)))
            if ln2_aff:
                ln2g_s = wpool.tile([P, H], f32, tag="ln2g")
                ln2b_s = wpool.tile([P, H], f32, tag="ln2b")
                nc.sync.dma_start(ln2g_s[:], ln2g_d[:].to_broadcast((P, H)))
                nc.sync.dma_start(ln2b_s[:], ln2b_d[:].to_broadcast((P, H)))

            def add_bias(ps, name, osl=None):
                """ps += broadcast(bias) via K=1 ones-matmul accumulation."""
                if name in bias_s:
                    b = bias_s[name]
                    src = b[:] if osl is None else b[:, osl]
                    nc.tensor.matmul(ps, ones_s[:], src, start=False, stop=True)

            # ---- per row-tile pipeline ----
            for t in range(NT):
                qt = inpool.tile([P, IB, P], bf16, tag="qt")
                q32 = inpool.tile([P, H], f32, tag="q32")
                kt = inpool.tile([P, C, IB, P], bf16, tag="kt")
                vt = inpool.tile([P, C, IB, P], bf16, tag="vt")
                nc.sync.dma_start(qt[:], qT_d[t])
                nc.sync.dma_start(q32[:], q32_d[t])
                nc.sync.dma_start(kt[:], kT_d[t])
                nc.sync.dma_start(vt[:], vT_d[t])

                # --- Q projection ---
                qp = pspool.tile([P, H], f32, tag="ps")
                for ib in range(IB):
                    nc.tensor.matmul(qp[:], qt[:, ib, :], wq_s[:, ib, :],
                                     start=(ib == 0), stop=(ib == IB - 1 and not use_bq))
                add_bias(qp[:], "bq")
                qs = small.tile([P, H], bf16, tag="qs")
                nc.scalar.copy(qs[:], qp[:])

                # --- K / V projections ---
                ks = proj.tile([P, C, H], bf16, tag="ks")
                vs = proj.tile([P, C, H], bf16, tag="vs")
                for c in range(C):
                    kp = pspool.tile([P, H], f32, tag="ps")
                    for ib in range(IB):
                        nc.tensor.matmul(kp[:], kt[:, c, ib, :], wk_s[:, ib, :],
                                         start=(ib == 0), stop=(ib == IB - 1 and not use_bk))
                    add_bias(kp[:], "bk")
                    nc.scalar.copy(ks[:, c, :], kp[:])
                for c in range(C):
                    vp = pspool.tile([P, H], f32, tag="ps")
                    for ib in range(IB):
                        nc.tensor.matmul(vp[:], vt[:, c, ib, :], wv_s[:, ib, :],
                                         start=(ib == 0), stop=(ib == IB - 1 and not use_bv))
                    add_bias(vp[:], "bv")
                    nc.scalar.copy(vs[:, c, :], vp[:])

                # --- scores: s[n, c, h] = sum_d Q[n, h*64+d] * K[n, c, h*64+d] ---
                pm = proj.tile([P, C, H], bf16, tag="pm")
                nc.vector.tensor_mul(pm[:], ks[:], qs[:, None, :].to_broadcast((P, C, H)))
                s = small.tile([P, C * NH], f32, tag="s")
                nc.vector.tensor_reduce(
                    s.rearrange("p (c h) -> p c h", c=C),
                    pm.rearrange("p c (h d) -> p c h d", h=NH),
                    axis=mybir.AxisListType.X, op=OP.add)
                # exp((s) / sqrt(dh)); dh=64 -> scale 1/8. No max-subtraction
                # needed: |s| stays small for this problem scale.
                e = small.tile([P, C * NH], bf16, tag="e")
                nc.scalar.activation(e[:], s[:], AF.Exp, bias=zero_s[:], scale=1.0 / np.sqrt(DH))
                z = small.tile([P, NH], f32, tag="z")
                nc.vector.tensor_reduce(
                    z[:], e.rearrange("p (c h) -> p h c", c=C),
                    axis=mybir.AxisListType.X, op=OP.add)
                zi = small.tile([P, NH], f32, tag="zi")
                nc.vector.reciprocal(zi[:], z[:])
                att = small.tile([P, C * NH], bf16, tag="att")
                nc.vector.tensor_mul(
                    att.rearrange("p (c h) -> p c h", c=C),
                    e.rearrange("p (c h) -> p c h", c=C),
                    zi[:, None, :].to_broadcast((P, C, NH)))

                # --- x[n, hd] = sum_c att[n, c, h] * V[n, c, hd] ---
                m = proj.tile([P, C, H], bf16, tag="m")
                nc.vector.tensor_mul(
                    m.rearrange("p c (h d) -> p c h d", h=NH),
                    vs.rearrange("p c (h d) -> p c h d", h=NH),
                    att.rearrange("p (c h) -> p c h", c=C)[:, :, :, None]
                       .to_broadcast((P, C, NH, DH)))
                x1 = small.tile([P, 4 * H], bf16, tag="x1")
                nc.vector.tensor_add(x1.rearrange("p (c h) -> p c h", c=4),
                                     m[:, 0:4, :], m[:, 4:8, :])
                x2 = small.tile([P, 2 * H], bf16, tag="x2")
                nc.vector.tensor_add(x2.rearrange("p (c h) -> p c h", c=2),
                                     x1[:, None, 0:2 * H].rearrange("p a (c h) -> p (a c) h", c=2),
                                     x1[:, None, 2 * H:4 * H].rearrange("p a (c h) -> p (a c) h", c=2))
                xb = small.tile([P, H], bf16, tag="xb")
                nc.vector.tensor_add(xb[:], x2[:, 0:H], x2[:, H:2 * H])

                # --- transpose x for O-projection (DMA xbar) ---
                xT = small.tile([P, IB, P], bf16, tag="xT")
                for ib in range(IB):
                    nc.sync.dma_start_transpose(xT[:, ib, :], xb[:, ib * P:(ib + 1) * P])

                # --- O projection + residual + LN1 ---
                op_ = pspool.tile([P, H], f32, tag="ps")
                for ib in range(IB):
                    nc.tensor.matmul(op_[:], xT[:, ib, :], wo_s[:, ib, :],
                                     start=(ib == 0), stop=(ib == IB - 1 and not use_bo))
                add_bias(op_[:], "bo")
                r1 = small.tile([P, H], f32, tag="r1")
                nc.vector.tensor_add(r1[:], q32[:], op_[:])

                def layernorm(src, dst, gs, bs, aff):
                    """dst = LN(src) (optionally affine). src f32 [P,H]."""
                    msum = small.tile([P, 1], f32, tag="msum")
                    nc.vector.reduce_sum(msum[:], src[:], axis=mybir.AxisListType.X)
                    mean = small.tile([P, 1], f32, tag="mean")
                    nc.vector.tensor_scalar_mul(mean[:], msum[:], 1.0 / H)
                    sqj = small.tile([P, H], f32, tag="sqj")
                    ssq = small.tile([P, 1], f32, tag="ssq")
                    nc.scalar.activation(sqj[:], src[:], AF.Square, bias=zero_s[:], accum_out=ssq[:])
                    m2 = small.tile([P, 1], f32, tag="m2")
                    nc.vector.tensor_mul(m2[:], mean[:], mean[:])
                    var = small.tile([P, 1], f32, tag="var")
                    nc.vector.scalar_tensor_tensor(
                        out=var[:], in0=ssq[:], scalar=1.0 / H, in1=m2[:],
                        op0=OP.mult, op1=OP.subtract)
                    sd = small.tile([P, 1], f32, tag="sd")
                    nc.scalar.activation(sd[:], var[:], AF.Sqrt, bias=eps_s[:])
                    rstd = small.tile([P, 1], f32, tag="rstd")
                    nc.vector.reciprocal(rstd[:], sd[:])
                    if aff:
                        tmpn = small.tile([P, H], f32, tag="tmpn")
                        nc.vector.tensor_scalar(
                            out=tmpn[:], in0=src[:], scalar1=mean[:], scalar2=rstd[:],
                            op0=OP.subtract, op1=OP.mult)
                        tmpg = small.tile([P, H], f32, tag="tmpg")
                        nc.vector.tensor_mul(tmpg[:], tmpn[:], gs[:])
                        nc.vector.tensor_add(dst, tmpg[:], bs[:])
                    else:
                        nc.vector.tensor_scalar(
                            out=dst, in0=src[:], scalar1=mean[:], scalar2=rstd[:],
                            op0=OP.subtract, op1=OP.mult)

                h1 = small.tile([P, H], bf16, tag="h1")
                layernorm(r1, h1[:], ln1g_s if ln1_aff else None,
                          ln1b_s if ln1_aff else None, ln1_aff)

                # --- FFN ---
                h1T = small.tile([P, IB, P], bf16, tag="h1T")
                for ib in range(IB):
                    nc.sync.dma_start_transpose(h1T[:, ib, :], h1[:, ib * P:(ib + 1) * P])
                f1a = pspool.tile([P, H], f32, tag="ps")
                f1b = pspool.tile([P, H], f32, tag="ps")
                for ib in range(IB):
                    nc.tensor.matmul(f1a[:], h1T[:, ib, :], w1_s[:, ib, 0:H],
                                     start=(ib == 0), stop=(ib == IB - 1 and not use_b1))
                for ib in range(IB):
                    nc.tensor.matmul(f1b[:], h1T[:, ib, :], w1_s[:, ib, H:HF],
                                     start=(ib == 0), stop=(ib == IB - 1 and not use_b1))
                if use_b1:
                    add_bias(f1a[:], "b1", osl=slice(0, H))
                    add_bias(f1b[:], "b1", osl=slice(H, HF))
                g = small.tile([P, HF], bf16, tag="g")
                nc.scalar.activation(g[:, 0:H], f1a[:], AF.Relu, bias=zero_s[:])
                nc.scalar.activation(g[:, H:HF], f1b[:], AF.Relu, bias=zero_s[:])
                gT = small.tile([P, JB, P], bf16, tag="gT")
                for jb in range(JB):
                    nc.sync.dma_start_transpose(gT[:, jb, :], g[:, jb * P:(jb + 1) * P])
                f2 = pspool.tile([P, H], f32, tag="ps")
                for jb in range(JB):
                    nc.tensor.matmul(f2[:], gT[:, jb, :], w2_s[:, jb, :],
                                     start=(jb == 0), stop=(jb == JB - 1 and not use_b2))
                add_bias(f2[:], "b2")

                # --- residual2 + LN2 -> output ---
                r2 = small.tile([P, H], f32, tag="r2")
                nc.vector.tensor_add(r2[:], h1[:], f2[:])
                outt = small.tile([P, H], f32, tag="outt")
                layernorm(r2, outt[:], ln2g_s if ln2_aff else None,
                          ln2b_s if ln2_aff else None, ln2_aff)
                nc.sync.dma_start(out_d[t], outt[:])

    nc.finalize()
    return nc


def _prep_inputs(inputs):
    """Host-side shard + reformat. Returns (in_maps, flags)."""
    import ml_dtypes
    bf = ml_dtypes.bfloat16

    q = np.ascontiguousarray(inputs["query"]).reshape(NROWS, H)
    k = np.ascontiguousarray(inputs["key"]).reshape(NROWS, C, H)
    v = np.ascontiguousarray(inputs["value"]).reshape(NROWS, C, H)

    def wT(w):  # [o, i] -> [p, ib, o] bf16 where i = ib*128+p
        return np.ascontiguousarray(
            w.T.reshape(IB, P, w.shape[0]).transpose(1, 0, 2)).astype(bf)

    wq, wk, wv, wo = (wT(np.asarray(inputs[n])) for n in ("Wq", "Wk", "Wv", "Wo"))
    w1 = wT(np.asarray(inputs["W1"]))  # [128, 4, 1024]
    w2 = np.ascontiguousarray(
        np.asarray(inputs["W2"]).T.reshape(JB, P, H).transpose(1, 0, 2)).astype(bf)

    flags = []
    biases = {}
    for n in ("bq", "bk", "bv", "bo", "b1", "b2"):
        key = n if n in inputs else n  # names match reference
        arr = np.asarray(inputs[key])
        used = bool(np.any(arr != 0))
        flags.append(used)
        if used:
            biases[n] = arr.reshape(1, -1).astype(bf)
    ln1_aff = bool(np.any(np.asarray(inputs["ln1_g"]) != 1) or np.any(np.asarray(inputs["ln1_b"]) != 0))
    ln2_aff = bool(np.any(np.asarray(inputs["ln2_g"]) != 1) or np.any(np.asarray(inputs["ln2_b"]) != 0))
    flags += [ln1_aff, ln2_aff]

    in_maps = []
    for ci in range(NCORES):
        rs = slice(ci * RPC, (ci + 1) * RPC)
        qs_ = q[rs]
        qT = np.ascontiguousarray(
            qs_.reshape(NT, P, IB, P).transpose(0, 3, 2, 1)).astype(bf)
        q32 = np.ascontiguousarray(qs_.reshape(NT, P, H)).astype(np.float32)
        kT = np.ascontiguousarray(
            k[rs].reshape(NT, P, C, IB, P).transpose(0, 4, 2, 3, 1)).astype(bf)
        vT = np.ascontiguousarray(
            v[rs].reshape(NT, P, C, IB, P).transpose(0, 4, 2, 3, 1)).astype(bf)
        im = {"qT": qT, "q32": q32, "kT": kT, "vT": vT,
              "wq": wq, "wk": wk, "wv": wv, "wo": wo, "w1": w1, "w2": w2}
        for n, arr in biases.items():
            im[n] = arr
        if ln1_aff:
            im["ln1g"] = np.asarray(inputs["ln1_g"]).reshape(1, H).astype(np.float32)
            im["ln1b"] = np.asarray(inputs["ln1_b"]).reshape(1, H).astype(np.float32)
        if ln2_aff:
            im["ln2g"] = np.asarray(inputs["ln2_g"]).reshape(1, H).astype(np.float32)
            im["ln2b"] = np.asarray(inputs["ln2_b"]).reshape(1, H).astype(np.float32)
        in_maps.append(im)
    return in_maps, tuple(flags)


def _run(in_maps, flags, trace=False, tmpdir=None):
    from concourse import bass_utils
    key = flags
    if key not in _CACHE:
        _CACHE[key] = _build(flags)
    nc = _CACHE[key]
    kwargs = {}
    if trace:
        kwargs = dict(trace=True, tmpdir=tmpdir)
    res = bass_utils.run_bass_kernel_spmd(nc, in_maps, core_ids=list(range(NCORES)), **kwargs)
    return res


def kernel(**inputs):
    in_maps, flags = _prep_inputs(inputs)
    res = _run(in_maps, flags)
    outs = [r["out"].reshape(RPC, H) for r in res.results]
    return np.concatenate(outs, axis=0).reshape(B, F, H).astype(np.float32)
